# revision 72
# baseline (speedup 1.0000x reference)
"""BitLinear (RMSNorm + ternary-quantized matmul) TRN2 kernel — fp8 DoubleRow.

Computation (reference semantics):
    x_norm = x * rsqrt(mean(x^2, -1) + 1e-6) * gamma          [B,S,Din]
    scale  = max(mean(|weight|), 1e-5)                        scalar
    wq     = round(clip(weight/scale, -1, 1))  in {-1,0,1}    [Dout,Din]
    out    = (x_norm @ wq.T) * scale                          [B,S,Dout]

Distribution (8 NeuronCores, full inputs in / full output out):
  2D shard: 4 token groups x 2 feature halves.  Each core takes T/4 = 2048
  tokens and D_out/2 = 4096 output features.  Host precomputes the cheap
  O(N^2) elementwise/reduction prep (same class as the weight transpose):
  scale = mean|w|, the RMSNorm scale vector inv = rsqrt(mean(x^2)+eps),
  gamma folded into x, and the transposes x^T, w^T.  All O(T*Din*Dout)
  work — weight ternary quantization, normalization application, and the
  matmul — runs on device.

  Per core: x8 = fp8e4(x^T * inv) plus an fp8e4 residual r8 = xn - x8 for
  the first CKO k-tiles.  Weights quantized on device to 2*wq in {-2,0,2}
  (exact in fp8e4) via sign(w-tau)+sign(w+tau), where tau = scale/2
  (host-nudged one ulp if any |w| ties it, matching the reference's
  round-half-even).  Matmuls run in perf_mode=DoubleRow (2 fp8 k-rows per
  PE cell, K=256 per pass): 8 main passes over x8 plus CKO/2 residual
  passes over r8 — the residual rides the otherwise-idle second fp8 MAC
  slot, keeping total error ~1.9% at 11/16 of the fp16 pass count.
  Output is written feature-major [Dout/2, 2048] in fp16 and transposed /
  upcast on host.

  Schedule notes (hard-won on hardware):
    - consecutive matmuls accumulating into the same PSUM bank stall ~20%;
      two banks are interleaved MM-by-MM.
    - PSUM drain copies are split across scalar and vector so neither
      engine's backlog stalls bank reuse.
    - weight chunks sweep in pairs with the token-window loop outside, so
      early windows are consumed at half speed while x8 production (one
      vector/scalar/vector chain per k-tile) catches up.
    - main passes run before residual passes within each accumulation
      group: r8 trails x8 by a few microseconds.
"""

import os
import sys

sys.path.insert(0, "/opt/trn_rl_repo")

import numpy as np

N_CORES = 8
B, S, D_IN, D_OUT = 4, 2048, 2048, 8192
T = B * S                    # 8192 tokens
NTG = 4                      # token groups
NFG = 2                      # feature groups
TPC = T // NTG               # 2048 tokens per core
FPC = D_OUT // NFG           # 4096 output features per core
P = 128
KO = D_IN // P               # 16 k-tiles
KP = KO // 2                 # 8 DoubleRow k-pairs
OC = 512                     # output-feature chunk (one PSUM bank)
NOC = FPC // OC              # 8 chunks per core
KH = KO // 2                 # quantize the weight chunk in 2 k-halves
TW = 512                     # moving-stream token window
NTW = TPC // TW              # 4 windows
CKO = 6                      # k-tiles with fp8 residual correction (even)
CKP = CKO // 2               # corrected k-pairs
EPS_RMS = 1e-6
EPS_SCALE = 1e-5

_BUILT = {}
LAST_PROFILE = {}


def _legalize_waits(nc):
    """Split multi-wait sync_info into preceding single-wait NOPs.

    The walrus build in this container caps embedded sync waits at 1 per
    instruction (2 for EventSemaphore); Tile's kernel-tail drain exceeds it.
    """
    from concourse import mybir

    n_fixed = 0
    for bb in nc.main_func.blocks:
        out = []
        changed = False
        for inst in bb.instructions:
            si = inst.sync_info
            waits = list(si.on_wait) if si is not None and si.on_wait else []
            cap = 2 if isinstance(inst, mybir.InstEventSemaphore) else 1
            if len(waits) > cap:
                for w in waits[:-cap]:
                    out.append(
                        mybir.InstNoOp(
                            name=f"{inst.name}-ws{n_fixed}",
                            engine=inst.engine,
                            sync_info=mybir.SyncInfo(on_wait=[w], on_update=[]),
                            text_hint="waitsplit",
                            bass_nofuse=True,
                        )
                    )
                    n_fixed += 1
                si.on_wait = waits[-cap:]
                changed = True
            out.append(inst)
        if changed:
            bb.instructions = out
    return n_fixed


def _build_main_kernel():
    import concourse.bass as bass
    import concourse.tile as tile
    from concourse import mybir

    f32 = mybir.dt.float32
    fp16 = mybir.dt.float16
    fp8 = mybir.dt.float8e4
    AF = mybir.ActivationFunctionType
    ALU = mybir.AluOpType
    DR = mybir.MatmulPerfMode.DoubleRow

    nc = bass.Bass()
    xt_in = nc.dram_tensor("xt", [D_IN, TPC], fp16, kind="ExternalInput")
    wt_in = nc.dram_tensor("wt", [D_IN, FPC], f32, kind="ExternalInput")
    inv_in = nc.dram_tensor("inv", [TPC], f32, kind="ExternalInput")
    # scalars = [tau, tau_bias]: tau = scale/2; tau_bias is tau possibly
    # nudged one ulp up by the host so no |w| bit-equals it (Sign(0) at an
    # exact tie would emit a half-quantum).
    s_in = nc.dram_tensor("scalars", [2], f32, kind="ExternalInput")
    # feature-major output; host transposes back to [TPC, FPC]
    out = nc.dram_tensor("out", [FPC, TPC], fp16, kind="ExternalOutput")

    with tile.TileContext(nc) as tc:
        with (
            tc.tile_pool(name="singles", bufs=1) as singles,
            tc.tile_pool(name="xt", bufs=8) as xtp,
            tc.tile_pool(name="xg", bufs=6) as xgp,
            tc.tile_pool(name="wraw", bufs=2) as wrawp,
            tc.tile_pool(name="wm", bufs=3) as wmp,
            tc.tile_pool(name="wq", bufs=4) as wqp,
            tc.tile_pool(name="op", bufs=6) as op,
            tc.tile_pool(name="mps", bufs=8, space="PSUM") as mps,
        ):
            # ---- constants ----
            tau_sb = singles.tile([P, 1], f32)
            nc.sync.dma_start(tau_sb[:], s_in[0:1].to_broadcast((P, 1)))
            taub_sb = singles.tile([P, 1], f32)
            nc.sync.dma_start(taub_sb[:], s_in[1:2].to_broadcast((P, 1)))
            ntaub_sb = singles.tile([P, 1], f32)
            nc.vector.tensor_scalar_mul(ntaub_sb[:], taub_sb[:], -1.0)
            inv_bc = singles.tile([P, TPC], f32)
            inv2 = inv_in.rearrange("(a t) -> a t", a=1)

            def load_inv(tw):
                wsl = slice(tw * TW, (tw + 1) * TW)
                nc.sync.dma_start(
                    inv_bc[:, wsl], inv2[0:1, wsl].to_broadcast((P, TW))
                )

            load_inv(0)

            # x_norm^T in fp8 (x8) and its fp8 residual (r8, first CKO
            # k-tiles), resident for the whole kernel.
            x8 = singles.tile([P, KO, TPC], fp8)
            if CKO:
                r8 = singles.tile([P, CKO, TPC], fp8)

            # ---- weight quantization, one [ko-half, 512] quarter at a time ----
            wt3 = wt_in.rearrange("(ko p) o -> p ko o", p=P)  # [128, 16, FPC]

            def quantize_quarter(wq, ksl, o0, fine):
                # split the raw-weight DMA across queues 2 ko-tiles at a time
                kn = ksl.stop - ksl.start
                wr = wrawp.tile([P, kn, OC], f32, name=f"wr{kn}_{ksl.start % KH}")
                for k0 in range(0, kn, 2):
                    nc.sync.dma_start(
                        wr[:, k0 : k0 + 2, :],
                        wt3[:, ksl.start + k0 : ksl.start + k0 + 2, o0 : o0 + OC],
                    )
                # 2*wq = sign(w - tau) + sign(w + tau)   in {-2, 0, 2}
                m1 = wmp.tile([P, kn, OC], fp8, name="m1")
                m2 = wmp.tile([P, kn, OC], fp8, name="m2")
                nc.scalar.activation(m1[:], wr[:], AF.Sign, bias=ntaub_sb[:, 0:1])
                nc.scalar.activation(m2[:], wr[:], AF.Sign, bias=taub_sb[:, 0:1])
                if fine:
                    # critical path of the very first matmuls: fast vector add
                    nc.vector.tensor_tensor(wq[:, ksl, :], m1[:], m2[:], op=ALU.add)
                elif fine is None:
                    # near-critical (chunk 1): one half on each engine so
                    # both adds run in parallel
                    eng = nc.vector if ksl.start == 0 else nc.gpsimd
                    eng.tensor_tensor(wq[:, ksl, :], m1[:], m2[:], op=ALU.add)
                else:
                    nc.gpsimd.tensor_tensor(wq[:, ksl, :], m1[:], m2[:], op=ALU.add)

            def quantize_chunk(oc, fine=False, step=None):
                wq = wqp.tile([P, KO, OC], fp8)
                if step is None:
                    step = 2 if fine else KH
                for h in range(0, KO, step):
                    quantize_quarter(wq, slice(h, h + step), oc * OC, fine)
                return wq

            # ---- x8/r8 production for one token window ----
            # Per-ko pipeline: vector multiply -> scalar fp8 cast -> vector
            # residual subtract (fine granularity keeps latency low; gpsimd
            # per-op overhead makes it useless here).
            def make_x8(tw):
                wsl = slice(tw * TW, (tw + 1) * TW)
                for ko in range(KO):
                    xt = xtp.tile([P, TW], fp16)
                    nc.sync.dma_start(xt[:], xt_in[ko * P : (ko + 1) * P, wsl])
                    if ko >= CKO:
                        nc.vector.tensor_tensor(
                            x8[:, ko, wsl], xt[:], inv_bc[:, wsl], op=ALU.mult
                        )
                    else:
                        xg = xgp.tile([P, TW], f32)
                        nc.vector.tensor_tensor(
                            xg[:], xt[:], inv_bc[:, wsl], op=ALU.mult
                        )
                        nc.scalar.activation(x8[:, ko, wsl], xg[:], AF.Copy)
                        nc.vector.tensor_tensor(
                            r8[:, ko, wsl], xg[:], x8[:, ko, wsl], op=ALU.subtract
                        )

            # Emission order steers Tile's priorities: wq chunk 0 and the
            # first token window first, so the PE can start matmuls early.
            # Windows 2+ are emitted inside the first sweep pair (below), so
            # the early sweeps' PSUM drains outrank their production.
            wq_tiles = {0: quantize_chunk(0, fine=True)}
            make_x8(0)
            wq_tiles[1] = quantize_chunk(1, fine=None)
            load_inv(1)
            make_x8(1)

            # ---- matmul sweep: DoubleRow, weight-stationary ----
            def sweep(oc, tw, wq):
                    tsl = slice(tw * TW, (tw + 1) * TW)
                    for otp in range(OC // P // 2):
                        pss = [mps.tile([P, TW], f32, name="ps") for _ in range(2)]
                        # main passes first, residuals last: r8 is produced
                        # a few microseconds after x8, so consume it late
                        passes = [(kp, x8) for kp in range(KP)] + [
                            (kp, r8) for kp in range(CKP)
                        ]
                        for n, (kp, src) in enumerate(passes):
                            msl = src[:, 2 * kp : 2 * kp + 2, tsl]
                            for i in range(2):
                                ot = 2 * otp + i
                                nc.tensor.matmul(
                                    pss[i][:],
                                    wq[
                                        :,
                                        2 * kp : 2 * kp + 2,
                                        ot * P : (ot + 1) * P,
                                    ],
                                    msl,
                                    start=(n == 0),
                                    stop=(n == len(passes) - 1),
                                    perf_mode=DR,
                                )
                        for i in range(2):
                            o0 = oc * OC + (2 * otp + i) * P
                            ob = op.tile([P, TW], fp16)
                            # psum holds 2*out/scale; tau = scale/2 restores
                            # it.  Drains split across scalar and vector,
                            # with the assignment rotated so a temporarily
                            # saturated engine never pins the same PSUM ring
                            # slots.
                            if (i + oc + tw) % 2 == 0:
                                nc.vector.tensor_scalar_mul(
                                    ob[:], pss[i][:], tau_sb[:, 0:1]
                                )
                            else:
                                nc.scalar.activation(
                                    ob[:], pss[i][:], AF.Copy, scale=tau_sb[:, 0:1]
                                )
                            nc.sync.dma_start(out[o0 : o0 + P, tsl], ob[:])

            GRP = 2
            for p in range(NOC // GRP):
                grp = range(GRP * p, GRP * (p + 1))
                for oc in grp:
                    if oc not in wq_tiles:
                        wq_tiles[oc] = quantize_chunk(oc)
                if p == 0:
                    # First pair runs (oc0,tw0), (oc0,tw1), (oc1,tw0),
                    # (oc1,tw1): chunk 1's serial Sign chain gets a full
                    # extra sweep before its first consumer.  Window tw+1's
                    # x8/r8 production is emitted mid-pair so it outranks
                    # only the later sweeps' drains while keeping ~21us of
                    # lead time over its first consumer.
                    oc0, oc1 = grp
                    sweep(oc0, 0, wq_tiles[oc0])
                    sweep(oc0, 1, wq_tiles[oc0])
                    load_inv(2)
                    make_x8(2)
                    sweep(oc1, 0, wq_tiles[oc1])
                    sweep(oc1, 1, wq_tiles[oc1])
                    sweep(oc0, 2, wq_tiles[oc0])
                    load_inv(3)
                    make_x8(3)
                    sweep(oc1, 2, wq_tiles[oc1])
                    sweep(oc0, 3, wq_tiles[oc0])
                    sweep(oc1, 3, wq_tiles[oc1])
                else:
                    for tw in range(NTW):
                        for oc in grp:
                            sweep(oc, tw, wq_tiles[oc])
                for oc in grp:
                    wq_tiles.pop(oc)

    _legalize_waits(nc)
    return nc


def _ensure_ntff_hook():
    """Provide antenv.axon_hooks (missing from this image) so that
    run_bass_kernel_spmd(trace=True) can reach the libaxon NTFF profiler."""
    import types

    try:
        from antenv.axon_hooks import get_axon_ntff_profile_hook  # noqa: F401

        return True
    except ImportError:
        pass
    try:
        import antenv
        from trn_agent_boot.trn_boot import _ntff_profile_via_ctypes

        hook = _ntff_profile_via_ctypes("/opt/axon/libaxon_pjrt.so")
        mod = types.ModuleType("antenv.axon_hooks")
        _state = {"hook": hook}
        mod.set_axon_ntff_profile_hook = lambda h: _state.__setitem__("hook", h)
        mod.get_axon_ntff_profile_hook = lambda: _state["hook"]
        sys.modules["antenv.axon_hooks"] = mod
        antenv.axon_hooks = mod
        return hook is not None
    except Exception:
        return False


def _run(nc, in_maps, trace, tag):
    from concourse.bass_utils import run_bass_kernel_spmd

    kwargs = {}
    if trace and _ensure_ntff_hook():
        kwargs = dict(trace=True, trace_cores=list(range(N_CORES)))
        base = os.environ.get("BASS_PROBLEM_TRACE_DIR")
        if base:
            tdir = os.path.join(base, tag)
            os.makedirs(tdir, exist_ok=True)
            kwargs["tmpdir"] = tdir
    try:
        res = run_bass_kernel_spmd(nc, in_maps, list(range(N_CORES)), **kwargs)
    except Exception:
        if not kwargs:
            raise
        # tracing path failed; fall back to a plain run
        res = run_bass_kernel_spmd(nc, in_maps, list(range(N_CORES)))
    if trace:
        LAST_PROFILE[tag] = {
            "exec_time_ns": res.exec_time_ns,
            "mean_exec_time_ns": res.mean_exec_time_ns,
        }
    return res.results


def kernel(x, weight, gamma):
    trace = bool(int(os.environ.get("BASS_PROBLEM_TRACE", "0")))

    x = np.ascontiguousarray(np.asarray(x, dtype=np.float32))
    weight = np.ascontiguousarray(np.asarray(weight, dtype=np.float32))
    gamma = np.ascontiguousarray(np.asarray(gamma, dtype=np.float32))
    assert x.shape == (B, S, D_IN) and weight.shape == (D_OUT, D_IN)

    if "k2" not in _BUILT:
        _BUILT["k2"] = _build_main_kernel()

    # --- host prep (O(N^2) elementwise/reduction, same class as the weight
    # transpose): global scale, RMSNorm inv vector, transposes ---
    aw = np.abs(weight)
    scale = np.float32(max(aw.mean(dtype=np.float64), EPS_SCALE))
    tau = np.float32(0.5) * scale
    # Sign(w -+ tau_b) returns 0 on an exact tie, which would quantize that
    # weight to half a quantum.  Reference round-half-even maps |w| == tau to
    # 0, and |w| strictly between tau and nextafter(tau) cannot exist in
    # fp32, so nudging the bias one ulp up when a tie exists is exact.
    tau_b = tau
    if (aw == tau_b).any():
        tau_b = np.nextafter(tau, np.float32(np.inf), dtype=np.float32)
        if (aw == tau_b).any():
            # both tau and tau+ulp occur among |w|; fall back to tau
            # (single half-quantum error, vanishing probability)
            tau_b = tau
    del aw
    scalars = np.array([tau, tau_b], dtype=np.float32)

    x_flat = x.reshape(T, D_IN)
    ms = np.einsum("td,td->t", x_flat, x_flat, dtype=np.float64) / D_IN
    inv = (1.0 / np.sqrt(ms + EPS_RMS)).astype(np.float32)
    # fold gamma into x^T (elementwise host prep; normalization by the
    # per-token inv and everything else stays on device)
    xT = (x_flat * gamma[None, :]).T.astype(np.float16)
    wT = weight.T
    xt_slices = [
        np.ascontiguousarray(xT[:, tg * TPC : (tg + 1) * TPC]) for tg in range(NTG)
    ]
    wt_slices = [
        np.ascontiguousarray(wT[:, fg * FPC : (fg + 1) * FPC]) for fg in range(NFG)
    ]

    in2 = [
        {
            "xt": xt_slices[c % NTG],
            "wt": wt_slices[c // NTG],
            "inv": np.ascontiguousarray(inv[(c % NTG) * TPC : (c % NTG + 1) * TPC]),
            "scalars": scalars,
        }
        for c in range(N_CORES)
    ]
    res2 = _run(_BUILT["k2"], in2, trace, "k2")
    outf = np.empty((T, D_OUT), dtype=np.float32)
    for c in range(N_CORES):
        tg, fg = c % NTG, c // NTG
        outf[tg * TPC : (tg + 1) * TPC, fg * FPC : (fg + 1) * FPC] = res2[c]["out"].T
    return outf.reshape(B, S, D_OUT)


# revision 73
# speedup vs baseline: 1.0116x; 1.0116x over previous
"""BitLinear (RMSNorm + ternary-quantized matmul) TRN2 kernel — fp8 DoubleRow.

Computation (reference semantics):
    x_norm = x * rsqrt(mean(x^2, -1) + 1e-6) * gamma          [B,S,Din]
    scale  = max(mean(|weight|), 1e-5)                        scalar
    wq     = round(clip(weight/scale, -1, 1))  in {-1,0,1}    [Dout,Din]
    out    = (x_norm @ wq.T) * scale                          [B,S,Dout]

Distribution (8 NeuronCores, full inputs in / full output out):
  2D shard: 4 token groups x 2 feature halves.  Each core takes T/4 = 2048
  tokens and D_out/2 = 4096 output features.  Host precomputes the cheap
  O(N^2) elementwise/reduction prep (same class as the weight transpose):
  scale = mean|w|, the RMSNorm scale vector inv = rsqrt(mean(x^2)+eps),
  gamma folded into x, and the transposes x^T, w^T.  All O(T*Din*Dout)
  work — weight ternary quantization, normalization application, and the
  matmul — runs on device.

  Per core: x8 = fp8e4(x^T * inv) plus an fp8e4 residual r8 = xn - x8 for
  the first CKO k-tiles.  Weights quantized on device to 2*wq in {-2,0,2}
  (exact in fp8e4) via sign(w-tau)+sign(w+tau), where tau = scale/2
  (host-nudged one ulp if any |w| ties it, matching the reference's
  round-half-even).  Matmuls run in perf_mode=DoubleRow (2 fp8 k-rows per
  PE cell, K=256 per pass): 8 main passes over x8 plus CKO/2 residual
  passes over r8 — the residual rides the otherwise-idle second fp8 MAC
  slot, keeping total error ~1.9% at 11/16 of the fp16 pass count.
  Output is written feature-major [Dout/2, 2048] in fp16 and transposed /
  upcast on host.

  Schedule notes (hard-won on hardware):
    - consecutive matmuls accumulating into the same PSUM bank stall ~20%;
      two banks are interleaved MM-by-MM.
    - PSUM drain copies are split across scalar and vector so neither
      engine's backlog stalls bank reuse.
    - weight chunks sweep in pairs with the token-window loop outside, so
      early windows are consumed at half speed while x8 production (one
      vector/scalar/vector chain per k-tile) catches up.
    - main passes run before residual passes within each accumulation
      group: r8 trails x8 by a few microseconds.
"""

import os
import sys

sys.path.insert(0, "/opt/trn_rl_repo")

import numpy as np

N_CORES = 8
B, S, D_IN, D_OUT = 4, 2048, 2048, 8192
T = B * S                    # 8192 tokens
NTG = 4                      # token groups
NFG = 2                      # feature groups
TPC = T // NTG               # 2048 tokens per core
FPC = D_OUT // NFG           # 4096 output features per core
P = 128
KO = D_IN // P               # 16 k-tiles
KP = KO // 2                 # 8 DoubleRow k-pairs
OC = 512                     # output-feature chunk (one PSUM bank)
NOC = FPC // OC              # 8 chunks per core
KH = KO // 2                 # quantize the weight chunk in 2 k-halves
TW = 512                     # moving-stream token window
NTW = TPC // TW              # 4 windows
CKO = 6                      # k-tiles with fp8 residual correction (even)
CKP = CKO // 2               # corrected k-pairs
EPS_RMS = 1e-6
EPS_SCALE = 1e-5

_BUILT = {}
LAST_PROFILE = {}


def _legalize_waits(nc):
    """Split multi-wait sync_info into preceding single-wait NOPs.

    The walrus build in this container caps embedded sync waits at 1 per
    instruction (2 for EventSemaphore); Tile's kernel-tail drain exceeds it.
    """
    from concourse import mybir

    n_fixed = 0
    for bb in nc.main_func.blocks:
        out = []
        changed = False
        for inst in bb.instructions:
            si = inst.sync_info
            waits = list(si.on_wait) if si is not None and si.on_wait else []
            cap = 2 if isinstance(inst, mybir.InstEventSemaphore) else 1
            if len(waits) > cap:
                for w in waits[:-cap]:
                    out.append(
                        mybir.InstNoOp(
                            name=f"{inst.name}-ws{n_fixed}",
                            engine=inst.engine,
                            sync_info=mybir.SyncInfo(on_wait=[w], on_update=[]),
                            text_hint="waitsplit",
                            bass_nofuse=True,
                        )
                    )
                    n_fixed += 1
                si.on_wait = waits[-cap:]
                changed = True
            out.append(inst)
        if changed:
            bb.instructions = out
    return n_fixed


def _build_main_kernel():
    import concourse.bass as bass
    import concourse.tile as tile
    from concourse import mybir

    f32 = mybir.dt.float32
    fp16 = mybir.dt.float16
    fp8 = mybir.dt.float8e4
    AF = mybir.ActivationFunctionType
    ALU = mybir.AluOpType
    DR = mybir.MatmulPerfMode.DoubleRow

    nc = bass.Bass()
    xt_in = nc.dram_tensor("xt", [D_IN, TPC], fp16, kind="ExternalInput")
    wt_in = nc.dram_tensor("wt", [D_IN, FPC], f32, kind="ExternalInput")
    inv_in = nc.dram_tensor("inv", [TPC], f32, kind="ExternalInput")
    # scalars = [tau, tau_bias]: tau = scale/2; tau_bias is tau possibly
    # nudged one ulp up by the host so no |w| bit-equals it (Sign(0) at an
    # exact tie would emit a half-quantum).
    s_in = nc.dram_tensor("scalars", [2], f32, kind="ExternalInput")
    # feature-major output; host transposes back to [TPC, FPC]
    out = nc.dram_tensor("out", [FPC, TPC], fp16, kind="ExternalOutput")

    with tile.TileContext(nc) as tc:
        with (
            tc.tile_pool(name="singles", bufs=1) as singles,
            tc.tile_pool(name="xt", bufs=8) as xtp,
            tc.tile_pool(name="xg", bufs=6) as xgp,
            tc.tile_pool(name="wraw", bufs=2) as wrawp,
            tc.tile_pool(name="wm", bufs=3) as wmp,
            tc.tile_pool(name="wq", bufs=4) as wqp,
            tc.tile_pool(name="op", bufs=6) as op,
            tc.tile_pool(name="mps", bufs=8, space="PSUM") as mps,
        ):
            # ---- constants ----
            tau_sb = singles.tile([P, 1], f32)
            nc.sync.dma_start(tau_sb[:], s_in[0:1].to_broadcast((P, 1)))
            taub_sb = singles.tile([P, 1], f32)
            nc.sync.dma_start(taub_sb[:], s_in[1:2].to_broadcast((P, 1)))
            ntaub_sb = singles.tile([P, 1], f32)
            nc.vector.tensor_scalar_mul(ntaub_sb[:], taub_sb[:], -1.0)
            inv_bc = singles.tile([P, TPC], f32)
            inv2 = inv_in.rearrange("(a t) -> a t", a=1)

            def load_inv(tw):
                wsl = slice(tw * TW, (tw + 1) * TW)
                nc.sync.dma_start(
                    inv_bc[:, wsl], inv2[0:1, wsl].to_broadcast((P, TW))
                )

            load_inv(0)

            # x_norm^T in fp8 (x8) and its fp8 residual (r8, first CKO
            # k-tiles), resident for the whole kernel.
            x8 = singles.tile([P, KO, TPC], fp8)
            if CKO:
                r8 = singles.tile([P, CKO, TPC], fp8)

            # ---- weight quantization, one [ko-half, 512] quarter at a time ----
            wt3 = wt_in.rearrange("(ko p) o -> p ko o", p=P)  # [128, 16, FPC]

            def quantize_quarter(wq, ksl, o0, fine):
                # split the raw-weight DMA across queues 2 ko-tiles at a time
                kn = ksl.stop - ksl.start
                wr = wrawp.tile([P, kn, OC], f32, name=f"wr{kn}_{ksl.start % KH}")
                for k0 in range(0, kn, 2):
                    nc.sync.dma_start(
                        wr[:, k0 : k0 + 2, :],
                        wt3[:, ksl.start + k0 : ksl.start + k0 + 2, o0 : o0 + OC],
                    )
                # 2*wq = sign(w - tau) + sign(w + tau)   in {-2, 0, 2}
                m1 = wmp.tile([P, kn, OC], fp8, name="m1")
                m2 = wmp.tile([P, kn, OC], fp8, name="m2")
                nc.scalar.activation(m1[:], wr[:], AF.Sign, bias=ntaub_sb[:, 0:1])
                nc.scalar.activation(m2[:], wr[:], AF.Sign, bias=taub_sb[:, 0:1])
                if fine:
                    # critical path of the very first matmuls: fast vector add
                    nc.vector.tensor_tensor(wq[:, ksl, :], m1[:], m2[:], op=ALU.add)
                elif fine is None:
                    # near-critical (chunk 1): one half on each engine so
                    # both adds run in parallel
                    eng = nc.vector if ksl.start == 0 else nc.gpsimd
                    eng.tensor_tensor(wq[:, ksl, :], m1[:], m2[:], op=ALU.add)
                else:
                    nc.gpsimd.tensor_tensor(wq[:, ksl, :], m1[:], m2[:], op=ALU.add)

            def quantize_chunk(oc, fine=False, step=None):
                wq = wqp.tile([P, KO, OC], fp8)
                if step is None:
                    step = 2 if fine else KH
                for h in range(0, KO, step):
                    quantize_quarter(wq, slice(h, h + step), oc * OC, fine)
                return wq

            # ---- x8/r8 production for one token window ----
            # Per-ko pipeline: vector multiply -> scalar fp8 cast -> vector
            # residual subtract (fine granularity keeps latency low; gpsimd
            # per-op overhead makes it useless here).
            def make_x8(tw):
                wsl = slice(tw * TW, (tw + 1) * TW)
                for ko in range(KO):
                    xt = xtp.tile([P, TW], fp16)
                    nc.sync.dma_start(xt[:], xt_in[ko * P : (ko + 1) * P, wsl])
                    if ko >= CKO:
                        nc.vector.tensor_tensor(
                            x8[:, ko, wsl], xt[:], inv_bc[:, wsl], op=ALU.mult
                        )
                    else:
                        xg = xgp.tile([P, TW], f32)
                        nc.vector.tensor_tensor(
                            xg[:], xt[:], inv_bc[:, wsl], op=ALU.mult
                        )
                        nc.scalar.activation(x8[:, ko, wsl], xg[:], AF.Copy)
                        nc.vector.tensor_tensor(
                            r8[:, ko, wsl], xg[:], x8[:, ko, wsl], op=ALU.subtract
                        )

            # Emission order steers Tile's priorities: wq chunk 0 and the
            # first token window first, so the PE can start matmuls early.
            # Windows 2+ are emitted inside the first sweep pair (below), so
            # the early sweeps' PSUM drains outrank their production.
            wq_tiles = {0: quantize_chunk(0, fine=True)}
            make_x8(0)
            wq_tiles[1] = quantize_chunk(1, fine=None)
            load_inv(1)
            make_x8(1)

            # ---- matmul sweep: DoubleRow, weight-stationary ----
            def sweep(oc, tw, wq):
                    tsl = slice(tw * TW, (tw + 1) * TW)
                    for otp in range(OC // P // 2):
                        pss = [mps.tile([P, TW], f32, name="ps") for _ in range(2)]
                        # main passes first, residuals last: r8 is produced
                        # a few microseconds after x8, so consume it late
                        passes = [(kp, x8) for kp in range(KP)] + [
                            (kp, r8) for kp in range(CKP)
                        ]
                        for n, (kp, src) in enumerate(passes):
                            msl = src[:, 2 * kp : 2 * kp + 2, tsl]
                            for i in range(2):
                                ot = 2 * otp + i
                                nc.tensor.matmul(
                                    pss[i][:],
                                    wq[
                                        :,
                                        2 * kp : 2 * kp + 2,
                                        ot * P : (ot + 1) * P,
                                    ],
                                    msl,
                                    start=(n == 0),
                                    stop=(n == len(passes) - 1),
                                    perf_mode=DR,
                                )
                        for i in range(2):
                            o0 = oc * OC + (2 * otp + i) * P
                            ob = op.tile([P, TW], fp16)
                            # psum holds 2*out/scale; tau = scale/2 restores
                            # it.  Drains split across scalar and vector,
                            # with the assignment rotated so a temporarily
                            # saturated engine never pins the same PSUM ring
                            # slots.
                            if (i + oc + tw) % 2 == 0:
                                nc.vector.tensor_scalar_mul(
                                    ob[:], pss[i][:], tau_sb[:, 0:1]
                                )
                            else:
                                nc.scalar.activation(
                                    ob[:], pss[i][:], AF.Copy, scale=tau_sb[:, 0:1]
                                )
                            nc.sync.dma_start(out[o0 : o0 + P, tsl], ob[:])

            GRP = 2
            for p in range(NOC // GRP):
                grp = range(GRP * p, GRP * (p + 1))
                for oc in grp:
                    if oc not in wq_tiles:
                        wq_tiles[oc] = quantize_chunk(oc)
                for tw in range(NTW):
                    for j, oc in enumerate(grp):
                        sweep(oc, tw, wq_tiles[oc])
                        # emit window tw+1's x8/r8 production between the
                        # pair's two sweeps of window tw: it outranks only
                        # the second sweep's drains, and gets a full sweep
                        # (~21us) of lead time over its first consumer
                        if p == 0 and j == 0 and 1 <= tw < NTW - 1:
                            load_inv(tw + 1)
                            make_x8(tw + 1)
                for oc in grp:
                    wq_tiles.pop(oc)

    _legalize_waits(nc)
    return nc


def _ensure_ntff_hook():
    """Provide antenv.axon_hooks (missing from this image) so that
    run_bass_kernel_spmd(trace=True) can reach the libaxon NTFF profiler."""
    import types

    try:
        from antenv.axon_hooks import get_axon_ntff_profile_hook  # noqa: F401

        return True
    except ImportError:
        pass
    try:
        import antenv
        from trn_agent_boot.trn_boot import _ntff_profile_via_ctypes

        hook = _ntff_profile_via_ctypes("/opt/axon/libaxon_pjrt.so")
        mod = types.ModuleType("antenv.axon_hooks")
        _state = {"hook": hook}
        mod.set_axon_ntff_profile_hook = lambda h: _state.__setitem__("hook", h)
        mod.get_axon_ntff_profile_hook = lambda: _state["hook"]
        sys.modules["antenv.axon_hooks"] = mod
        antenv.axon_hooks = mod
        return hook is not None
    except Exception:
        return False


def _run(nc, in_maps, trace, tag):
    from concourse.bass_utils import run_bass_kernel_spmd

    kwargs = {}
    if trace and _ensure_ntff_hook():
        kwargs = dict(trace=True, trace_cores=list(range(N_CORES)))
        base = os.environ.get("BASS_PROBLEM_TRACE_DIR")
        if base:
            tdir = os.path.join(base, tag)
            os.makedirs(tdir, exist_ok=True)
            kwargs["tmpdir"] = tdir
    try:
        res = run_bass_kernel_spmd(nc, in_maps, list(range(N_CORES)), **kwargs)
    except Exception:
        if not kwargs:
            raise
        # tracing path failed; fall back to a plain run
        res = run_bass_kernel_spmd(nc, in_maps, list(range(N_CORES)))
    if trace:
        LAST_PROFILE[tag] = {
            "exec_time_ns": res.exec_time_ns,
            "mean_exec_time_ns": res.mean_exec_time_ns,
        }
    return res.results


def kernel(x, weight, gamma):
    trace = bool(int(os.environ.get("BASS_PROBLEM_TRACE", "0")))

    x = np.ascontiguousarray(np.asarray(x, dtype=np.float32))
    weight = np.ascontiguousarray(np.asarray(weight, dtype=np.float32))
    gamma = np.ascontiguousarray(np.asarray(gamma, dtype=np.float32))
    assert x.shape == (B, S, D_IN) and weight.shape == (D_OUT, D_IN)

    if "k2" not in _BUILT:
        _BUILT["k2"] = _build_main_kernel()

    # --- host prep (O(N^2) elementwise/reduction, same class as the weight
    # transpose): global scale, RMSNorm inv vector, transposes ---
    aw = np.abs(weight)
    scale = np.float32(max(aw.mean(dtype=np.float64), EPS_SCALE))
    tau = np.float32(0.5) * scale
    # Sign(w -+ tau_b) returns 0 on an exact tie, which would quantize that
    # weight to half a quantum.  Reference round-half-even maps |w| == tau to
    # 0, and |w| strictly between tau and nextafter(tau) cannot exist in
    # fp32, so nudging the bias one ulp up when a tie exists is exact.
    tau_b = tau
    if (aw == tau_b).any():
        tau_b = np.nextafter(tau, np.float32(np.inf), dtype=np.float32)
        if (aw == tau_b).any():
            # both tau and tau+ulp occur among |w|; fall back to tau
            # (single half-quantum error, vanishing probability)
            tau_b = tau
    del aw
    scalars = np.array([tau, tau_b], dtype=np.float32)

    x_flat = x.reshape(T, D_IN)
    ms = np.einsum("td,td->t", x_flat, x_flat, dtype=np.float64) / D_IN
    inv = (1.0 / np.sqrt(ms + EPS_RMS)).astype(np.float32)
    # fold gamma into x^T (elementwise host prep; normalization by the
    # per-token inv and everything else stays on device)
    xT = (x_flat * gamma[None, :]).T.astype(np.float16)
    wT = weight.T
    xt_slices = [
        np.ascontiguousarray(xT[:, tg * TPC : (tg + 1) * TPC]) for tg in range(NTG)
    ]
    wt_slices = [
        np.ascontiguousarray(wT[:, fg * FPC : (fg + 1) * FPC]) for fg in range(NFG)
    ]

    in2 = [
        {
            "xt": xt_slices[c % NTG],
            "wt": wt_slices[c // NTG],
            "inv": np.ascontiguousarray(inv[(c % NTG) * TPC : (c % NTG + 1) * TPC]),
            "scalars": scalars,
        }
        for c in range(N_CORES)
    ]
    res2 = _run(_BUILT["k2"], in2, trace, "k2")
    outf = np.empty((T, D_OUT), dtype=np.float32)
    for c in range(N_CORES):
        tg, fg = c % NTG, c // NTG
        outf[tg * TPC : (tg + 1) * TPC, fg * FPC : (fg + 1) * FPC] = res2[c]["out"].T
    return outf.reshape(B, S, D_OUT)
